# revision 1
# baseline (speedup 1.0000x reference)
"""Trainium2 Bass kernel for EnhancedLIFWithMemory.

Model (per timestep t, per (batch, hidden) element):
    mask = (refrac_timer <= 0)  -> product of "no-spike" flags of the
                                   last ceil(ref_steps) steps
    syn  = spikes[:, t] @ W + b
    i    = a_syn * i + syn * mask
    v    = a_mem * v + i
    s    = (v > 1 + adapt)
    v    = v - s * (1 + adapt) * 0.8
    adapt= a_adapt * adapt + 0.1 * s

Sharding: data-parallel over batch (8 batches per core, 8 cores).

Per core, per 32-step block: the PE computes u = x @ W in fp32 into
PSUM (hidden tile k on partitions), ACT copies it (+bias) into SBUF,
and the scan runs the recurrence on [128, 64] state tiles (free =
(k, batch)).  The serial chain is 5 custom DVE ops per step (um, i, v,
s, reset); every cross-engine input is produced with >= 1 step of
slack so the DVE never sees an unsatisfied semaphore:
  - Pool computes z(t+1) = u(t+1) * (1 - s(t-1)) right after s(t-1)
    (one step before the DVE needs it),
  - adapt is kept pre-scaled (atilde = adapt / 0.1) so its update is
    one ACT mul (early) plus one Pool add that lands ~480ns after
    s(t-1), well before the spike op consumes it at ~+660ns.

Host side pre-transposes spikes to [block, d_tile, d_lo, tau*8+b] and
post-transposes the device output [block, h_lo, (tau, k, b)] to
[B, T, H].
"""

import os
import numpy as np

import concourse.bacc as bacc
import concourse.mybir as mybir
import concourse.tile as tile
from concourse.bass_utils import run_bass_kernel_spmd

DT = 1e-3
THRESHOLD = 1.0
RESET_FACTOR = 0.8
ADAPT_INCREMENT = 0.1

N_CORES = 8
B, T, D, H = 64, 512, 1024, 1024
TB = 32                    # timesteps per block
NBLK = T // TB             # 16 blocks
BPC = B // N_CORES         # 8 batches per core
NK = H // 128              # 8 hidden tiles
ND = D // 128              # 8 contraction tiles
NFREE = NK * BPC           # 64 = free size of state tiles
TCH = 2 * TB               # 64 = timesteps per matmul chunk (2 blocks)
NMM = TCH * BPC            # 512 = moving rows per matmul
UB = TB * NFREE            # 2048 = free size of per-block s tiles
UCH = TCH * NFREE          # 4096 = free size of per-chunk u tiles


def _f32(x):
    return float(np.float32(x))


def compute_scalars(log_tau_mem, log_tau_syn, log_tau_ref, log_tau_adapt):
    """Compute decay factors exactly as the (CPU jax) reference does."""
    try:
        import jax
        cpu = jax.local_devices(backend="cpu")[0]
        with jax.default_device(cpu):
            import jax.numpy as jnp
            a_mem = np.float32(jnp.exp(-DT / jnp.exp(jnp.asarray(log_tau_mem))))
            a_syn = np.float32(jnp.exp(-DT / jnp.exp(jnp.asarray(log_tau_syn))))
            a_adp = np.float32(jnp.exp(-DT / jnp.exp(jnp.asarray(log_tau_adapt))))
            ref_steps = np.float32(jnp.exp(jnp.asarray(log_tau_ref)) / DT)
    except Exception:
        f = np.float32
        a_mem = np.exp(f(-DT) / np.exp(f(log_tau_mem), dtype=f), dtype=f)
        a_syn = np.exp(f(-DT) / np.exp(f(log_tau_syn), dtype=f), dtype=f)
        a_adp = np.exp(f(-DT) / np.exp(f(log_tau_adapt), dtype=f), dtype=f)
        ref_steps = np.exp(f(log_tau_ref), dtype=f) / f(DT)
    w = int(np.ceil(float(ref_steps)))
    w = max(0, min(w, 2))
    return float(a_mem), float(a_syn), float(a_adp), w


_LIF_OPS = {}


def _register_dve_ops():
    """Register the custom fused DVE ops (idempotent)."""
    if _LIF_OPS:
        return _LIF_OPS
    from concourse.dve_spec import Spec, Src0, Src1, C0, C1, Zero, One, select, lower
    from concourse.dve_spec import _has_src1 as has_src1
    from concourse.dve_uop import DveOpSpec
    from concourse import dve_ops
    from concourse.dve_ops import DveOp, OPS, get_dve_sub_opcode

    def _make(name, spec):
        for o in OPS:
            if o.name == name:
                return o
        op = DveOp(name, spec, subdim=False, uops_sha={})
        OPS.append(op)
        dve_ops._SUB_OPCODE_FOR_NAME[name] = (
            dve_ops._CUSTOM_DVE_ROW_BASE + len(OPS) - 1)
        assert dve_ops._SUB_OPCODE_FOR_NAME[name] < 0x20
        for ver in ("v3",):
            compiled = DveOpSpec(
                name=op.name,
                opcode=get_dve_sub_opcode(op.name),
                uops=lower(op.spec, ver=ver),
                rd1_en=has_src1(op.spec),
            )
            op.uops_sha[ver] = compiled.sha(ver)
        return op

    _LIF_OPS["maskmul"] = _make(
        "LIF_MASKMUL",
        Spec(body=Src0 - Src0 * Src1,
             reference=lambda in0, in1, c0, c1, c2: in0 - in0 * in1))
    _LIF_OPS["axpy"] = _make(
        "LIF_AXPY",
        Spec(body=C0 * Src0 + Src1,
             reference=lambda in0, in1, c0, c1, c2: (
                 (np.float32(c0) * in0).astype(np.float32) + in1)))
    # e = 1 + C0 * atilde  (atilde = adapt / ADAPT_INCREMENT, C0 = 0.1)
    _es = C0 * Src1 + One
    _LIF_OPS["spike_sc"] = _make(
        "LIF_SPIKE_SC",
        Spec(body=Src0 > _es,
             reference=lambda in0, in1, c0, c1, c2: (
                 in0 > (np.float32(c0) * in1 + np.float32(1.0))
                 ).astype(np.float32)))

    def _reset_sc_ref(in0, in1, c0, c1, c2):
        e = (np.float32(c0) * in1).astype(np.float32) + np.float32(1.0)
        r = in0 - (np.float32(c1) * e).astype(np.float32)
        return np.where(in0 > e, r, in0).astype(np.float32)

    _LIF_OPS["reset_sc"] = _make(
        "LIF_RESET_SC",
        Spec(body=select(Src0 > _es, Src0 - C1 * _es, Src0),
             reference=_reset_sc_ref))
    return _LIF_OPS


def build_kernel(a_mem, a_syn, a_adp, wmask, with_bias, nblk=NBLK):
    ops = _register_dve_ops()
    Alu = mybir.AluOpType
    f32 = mybir.dt.float32
    nc = bacc.Bacc()

    assert nblk % 2 == 0
    nch = nblk // 2
    xT = nc.dram_tensor("xT", [nch, ND, 128, NMM], f32, kind="ExternalInput")
    Wt = nc.dram_tensor("Wt", [D, H], f32, kind="ExternalInput")
    bias = nc.dram_tensor("bias", [H], f32, kind="ExternalInput")
    y = nc.dram_tensor("y", [nblk, 128, UB], f32, kind="ExternalOutput")

    with tile.TileContext(nc) as tc:
        with (
            tc.tile_pool(name="wpool", bufs=1) as wpool,
            tc.tile_pool(name="spool", bufs=1) as spool,
            tc.tile_pool(name="xpool", bufs=2) as xpool,
            tc.tile_pool(name="upool", bufs=3) as upool,
            tc.tile_pool(name="opool", bufs=4) as opool,
            tc.tile_pool(name="tpool", bufs=6) as tpool,
            tc.tile_pool(name="pspool", bufs=6, space="PSUM") as pspool,
        ):
            wsb = [wpool.tile([128, H], f32, name=f"wsb{d}") for d in range(ND)]
            nc.sync.dma_start(wsb[0][:], Wt[0:128, :])

            # PE warm-up: ~8 throwaway matmuls on a zeroed scratch tile,
            # overlapping the first x-DMA.  The PE clock reaches full rate
            # only after ~3us of continuous execution; without this, the
            # whole first chunk (and the start of the second) runs at the
            # mid p-state and the scan hits a ~12us stall at step 63
            # waiting for chunk 1's u.
            warm = wpool.tile([128, NMM], f32, name="warm")
            nc.gpsimd.memset(warm[:], 0.0)
            for _wi in range(2):
                wps = pspool.tile([128, NMM], f32, name="ups")
                nc.tensor.matmul(wps[:], warm[:, 0:128], warm[:],
                                 start=True, stop=True)
            if with_bias:
                bias_sb = wpool.tile([128, NK], f32)
                nc.sync.dma_start(
                    bias_sb[:], bias[:].rearrange("(k p) -> p k", p=128))

            # Persistent scan state. The A/B column halves live in
            # SEPARATE tiles, each with a never-written padding half:
            # the serial chain's same-engine read-deps get their
            # dependency tracking pointed at the padding, so the Tile
            # layer emits no semaphore for them (the in-order engine
            # pipeline provides the RAW ordering; verified bit-exact on
            # hardware).  Writes keep truthful tracking so cross-engine
            # consumers (Pool/ACT/DMA) still synchronize.
            NH = NFREE // 2
            i_sts = [spool.tile([128, NH], f32, name=f"i_st{h}")
                     for h in (0, 1)]
            v_sts = [spool.tile([128, NH], f32, name=f"v_st{h}")
                     for h in (0, 1)]
            a_st = spool.tile([128, NFREE], f32)   # atilde = adapt / 0.1
            s_init = spool.tile([128, 2 * NFREE], f32)
            for h in (0, 1):
                nc.vector.memset(i_sts[h][:], 0.0)
                nc.vector.memset(v_sts[h][:], 0.0)
            nc.vector.memset(a_st[:], 0.0)
            nc.vector.memset(s_init[:], 0.0)

            s_blocks = {}   # blk -> s_sb tile
            z_tiles = {}    # t -> z tile (masked-input staging, Pool-made)
            a1_tiles = {}   # t -> a1 tile (ACT: a_adp * atilde(t-1))

            def s_hist(t_abs):
                """AP of the spike raster at absolute step t_abs."""
                if t_abs < 0:
                    return s_init[:, (t_abs % 2) * NFREE:][:, :NFREE]
                blk, tau = divmod(t_abs, TB)
                return s_blocks[blk][:, tau * NFREE:(tau + 1) * NFREE]

            def u_of(t_abs, u_tiles, half=None):
                """AP of the (bias-added) synaptic drive at step t_abs.

                u is stored [p, k, t64, b] so the PSUM->SBUF DMA per
                (k, pass) is contiguous; a step slice is [p, NK, BPC]
                strided."""
                ch, t64 = divmod(t_abs, TCH)
                u4 = u_tiles[ch][:].rearrange(
                    "p (k t b) -> p k t b", k=NK, b=BPC)
                if half is None:
                    return u4[:, :, t64, :]
                return u4[:, half * (NK // 2):(half + 1) * (NK // 2), t64, :]

            u_tiles = {}    # ch -> u tile

            def make_z(t_abs):
                """Pool: z(t) = u(t) * (1 - s(t-2)); for wmask<=1 z = u."""
                if t_abs >= nblk * TB:
                    return
                u_t = u_of(t_abs, u_tiles)          # [p, NK, BPC] strided
                if wmask <= 1:
                    z = tpool.tile([128, NFREE], f32, name="z")
                    nc.gpsimd.tensor_copy(
                        z[:].rearrange("p (k b) -> p k b", b=BPC), u_t)
                    z_tiles[t_abs] = z
                    return
                sh = s_hist(t_abs - 2).rearrange("p (k b) -> p k b", b=BPC)
                zz = tpool.tile([128, NFREE], f32, name="zz")
                nc.gpsimd.tensor_tensor(
                    zz[:].rearrange("p (k b) -> p k b", b=BPC), u_t, sh,
                    op=Alu.mult)
                z = tpool.tile([128, NFREE], f32, name="z")
                nc.gpsimd.tensor_tensor(
                    z[:].rearrange("p (k b) -> p k b", b=BPC), u_t,
                    zz[:].rearrange("p (k b) -> p k b", b=BPC),
                    op=Alu.subtract)
                z_tiles[t_abs] = z

            def z_ap(t_abs):
                return z_tiles.pop(t_abs)[:]

            def make_a1(t_abs):
                """ACT: a1(t) = a_adp * atilde(t-1)  (atilde after t-1 add)."""
                a1 = tpool.tile([128, NFREE], f32, name="a1")
                nc.scalar.mul(a1[:], a_st[:], _f32(a_adp))
                a1_tiles[t_abs] = a1

            pending_copies = []

            def emit_mm(ch, defer_copies=False):
                """Queue x-DMA + matmuls + PSUM->SBUF copies for chunk ch."""
                xsb = [xpool.tile([128, NMM], f32, name=f"xsb{d}")
                       for d in range(ND)]
                for d in range(ND):
                    nc.sync.dma_start(xsb[d][:], xT[ch, d])
                if ch == 0:
                    # remaining W tiles after the first x chunk is queued
                    for d in range(1, ND):
                        nc.sync.dma_start(wsb[d][:], Wt[d * 128:(d + 1) * 128, :])

                # u for this 64-step chunk, [p, k * (TCH*BPC) + t64*BPC + b]
                u_sb = upool.tile([128, UCH], f32, name="u_sb")
                u_tiles[ch] = u_sb
                u4 = u_sb[:].rearrange("p (k t b) -> p k t b", k=NK, b=BPC)
                # Chunk 0: four quarter-width passes so the scan starts
                # early.  Chunk 1: two half-width passes so the scan's
                # step-64 deadline only needs the first-half PSUMs (the PE
                # cannot finish two full chunks before the scan reaches
                # step 64 otherwise).
                halves = (4 if ch == 0 else (2 if ch == 1 else 1))
                hn = NMM // halves
                for h in range(halves):
                    for k in range(NK):
                        ups = pspool.tile([128, hn], f32, name="ups")
                        for d in range(ND):
                            nc.tensor.matmul(
                                ups[:],
                                wsb[d][:, k * 128:(k + 1) * 128],
                                xsb[d][:, h * hn:(h + 1) * hn],
                                start=(d == 0),
                                stop=(d == ND - 1),
                            )
                        tn = TCH // halves
                        dst = u4[:, k, h * tn:(h + 1) * tn, :]
                        src = ups[:].rearrange("p (t b) -> p t b", b=BPC)
                        # Stagger the copies into the upcoming scan window
                        # (negative priority offset = apparent later issue)
                        # paced to when the PE finishes each PSUM tile, so
                        # the per-step a1 muls never queue behind a burst
                        # of copies on the in-order ACT engine.
                        if defer_copies:
                            # emitted later, inside the previous chunk's tau
                            # loop, paced to PE completion of each PSUM
                            pending_copies.append((dst, src, k))
                            continue
                        off = None if ch == 0 else -(190 + 107 * k)
                        with tc.high_priority(offset=off):
                            if with_bias:
                                nc.scalar.activation(
                                    dst, src,
                                    mybir.ActivationFunctionType.Identity,
                                    bias=bias_sb[:, k:k + 1], scale=1.0)
                            else:
                                nc.scalar.copy(dst, src)

            for ch in range(nch):
                if ch == 0:
                    emit_mm(0)
                if ch + 1 < nch:
                    # PE runs one full chunk ahead of the scan.  Chunk 1's
                    # copies are deferred into chunk 0's tau loop (its PE
                    # work starts late, after the first chunk's ramp; the
                    # usual priority stagger would reach back into mm(0)'s
                    # emission range and head-of-line block the ACT queue).
                    emit_mm(ch + 1, defer_copies=(ch == 0))

                if ch == 0:
                    # Prologue: z for the first two steps (s(-2)=s(-1)=0) and
                    # the first a1.
                    make_z(0)
                    make_z(1)
                    make_a1(0)

                for blk in (2 * ch, 2 * ch + 1):
                    s_sb = opool.tile([128, UB], f32)
                    s_blocks[blk] = s_sb

                    for tau in range(TB):
                        t = blk * TB + tau
                        s_t = s_sb[:, tau * NFREE:(tau + 1) * NFREE]

                        # --- DVE serial chain: 5 logical ops, split into
                        # A/B column halves (separate tiles) and
                        # interleaved so every RAW dependency is >= 2
                        # instructions back (the engine then streams at
                        # ~100ns/op instead of paying a ~200ns semaphore-
                        # resolution tax per hazard). ---
                        hs = (slice(0, NH), slice(NH, NFREE))
                        zt = z_ap(t)
                        a_rd = a_st
                        if wmask == 0:
                            um_aps = (zt[:, hs[0]], zt[:, hs[1]])
                        else:
                            ums = [tpool.tile([128, NH], f32, name=f"um{h}")
                                   for h in (0, 1)]
                            sp = s_hist(t - 1)
                            for h in (0, 1):
                                nc.vector._custom_dve(
                                    ops["maskmul"], out=ums[h][:],
                                    in0=zt[:, hs[h]], in1=sp[:, hs[h]])
                            um_aps = (ums[0][:], ums[1][:])

                        def emit_i(h):
                            nc.vector._custom_dve(
                                ops["axpy"], out=i_sts[h][:],
                                in0=i_sts[h][:],
                                in1=um_aps[h], s0=_f32(a_syn))

                        def emit_v(h):
                            nc.vector._custom_dve(
                                ops["axpy"], out=v_sts[h][:],
                                in0=v_sts[h][:],
                                in1=i_sts[h][:], s0=_f32(a_mem))

                        def emit_s(h):
                            nc.vector._custom_dve(
                                ops["spike_sc"], out=s_t[:, hs[h]],
                                in0=v_sts[h][:],
                                in1=a_rd[:, hs[h]], s0=_f32(ADAPT_INCREMENT))

                        def emit_r(h):
                            nc.vector._custom_dve(
                                ops["reset_sc"], out=v_sts[h][:],
                                in0=v_sts[h][:],
                                in1=a_rd[:, hs[h]],
                                s0=_f32(ADAPT_INCREMENT),
                                s1=_f32(RESET_FACTOR))

                        emit_i(0)
                        emit_i(1)
                        emit_v(0)
                        emit_v(1)
                        emit_s(0)
                        emit_s(1)
                        emit_r(0)
                        emit_r(1)

                        # --- Pool burst (all depend only on s(t)) ---
                        # atilde(t) = a1(t) + s(t), into the t%2 buffer
                        a1 = a1_tiles.pop(t)
                        nc.gpsimd.tensor_tensor(a_st[:], a1[:], s_t,
                                                op=Alu.add)
                        # z(t+2) = u(t+2) * (1 - s(t)); one step of slack
                        # before the DVE consumes it at step t+2... emitted
                        # here so Pool order is add, zz, z.
                        make_z(t + 2)
                        # ACT: a1(t+1) = a_adp * atilde(t)
                        make_a1(t + 1)
                        # chunk-1 copies, paced to when the PE finishes each
                        # PSUM (~6.8us apart starting ~34us in)
                        if pending_copies and t in (
                                22, 25, 28, 31, 34, 37, 40, 43,
                                57, 59, 61, 63, 65, 67, 69, 71):
                            dst_, src_, k_ = pending_copies.pop(0)
                            if with_bias:
                                nc.scalar.activation(
                                    dst_, src_,
                                    mybir.ActivationFunctionType.Identity,
                                    bias=bias_sb[:, k_:k_ + 1], scale=1.0)
                            else:
                                nc.scalar.copy(dst_, src_)

                    if blk == nblk - 1:
                        # final block: stream the output in quarters so the
                        # last DMA is not exposed after the scan ends
                        for pc_ in range(4):
                            p0_ = pc_ * (UB // 4)
                            p1_ = (pc_ + 1) * (UB // 4)
                            nc.sync.dma_start(
                                y[blk, :, p0_:p1_], s_sb[:, p0_:p1_])
                    else:
                        nc.sync.dma_start(y[blk], s_sb[:])
                    if blk >= 2:
                        s_blocks.pop(blk - 2, None)
                if ch >= 1:
                    u_tiles.pop(ch - 1, None)

    nc.compile()
    return nc


def _install_ntff_shim():
    """The container's antenv package lacks axon_hooks; recreate the NTFF
    profile hook (ctypes into libaxon_pjrt.so) so trace=True works."""
    import sys
    if "antenv.axon_hooks" in sys.modules:
        return
    import contextlib
    import ctypes
    import types

    so_path = "/opt/axon/libaxon_pjrt.so"
    hook = None
    if os.path.exists(so_path):
        lib = ctypes.CDLL(so_path)
        if hasattr(lib, "axon_start_nrt_profile"):
            lib.axon_start_nrt_profile.argtypes = [
                ctypes.POINTER(ctypes.c_int64), ctypes.c_size_t]
            lib.axon_start_nrt_profile.restype = ctypes.c_int64
            lib.axon_stop_nrt_profile.argtypes = [ctypes.c_char_p]
            lib.axon_stop_nrt_profile.restype = ctypes.c_int64

            @contextlib.contextmanager
            def _hook(output_dir, device_ids):
                import jax
                jax.devices()
                if device_ids:
                    ids = (ctypes.c_int64 * len(device_ids))(*device_ids)
                    rc = lib.axon_start_nrt_profile(ids, len(device_ids))
                else:
                    rc = lib.axon_start_nrt_profile(None, 0)
                if rc != 0:
                    raise RuntimeError(f"axon_start_nrt_profile rc={rc}")
                try:
                    yield
                finally:
                    n = lib.axon_stop_nrt_profile(str(output_dir).encode())
                    if n < 0:
                        raise RuntimeError(f"axon_stop_nrt_profile rc={n}")

            hook = _hook

    mod = types.ModuleType("antenv.axon_hooks")
    mod.get_axon_ntff_profile_hook = lambda: hook
    mod.set_axon_ntff_profile_hook = lambda h: None
    sys.modules["antenv.axon_hooks"] = mod


_CACHE = {}


def _get_kernel(key, *args):
    if key not in _CACHE:
        _CACHE[key] = build_kernel(*args)
    return _CACHE[key]


def kernel(spikes, W, b, log_tau_mem, log_tau_syn, log_tau_ref, log_tau_adapt,
           _trace=False):
    spikes = np.ascontiguousarray(np.asarray(spikes, dtype=np.float32))
    W = np.ascontiguousarray(np.asarray(W, dtype=np.float32))
    b = np.ascontiguousarray(np.asarray(b, dtype=np.float32))
    a_mem, a_syn, a_adp, wmask = compute_scalars(
        np.asarray(log_tau_mem), np.asarray(log_tau_syn),
        np.asarray(log_tau_ref), np.asarray(log_tau_adapt))
    with_bias = bool(np.any(b))

    if _trace:
        _install_ntff_shim()

    nc = _get_kernel((a_mem, a_syn, a_adp, wmask, with_bias),
                     a_mem, a_syn, a_adp, wmask, with_bias)

    # Host-side shard + transpose: [8, 512, 1024] -> [8, 8, 128, 512]
    nch = NBLK // 2
    in_maps = []
    for c in range(N_CORES):
        xc = spikes[c * BPC:(c + 1) * BPC]           # [8, 512, 1024]
        xc = xc.reshape(BPC, nch, TCH, ND, 128)
        xTc = np.ascontiguousarray(xc.transpose(1, 3, 4, 2, 0)).reshape(
            nch, ND, 128, NMM)
        in_maps.append({"xT": xTc, "Wt": W, "bias": b})

    for attempt in range(4):
        try:
            res = run_bass_kernel_spmd(
                nc, in_maps, core_ids=list(range(N_CORES)),
                trace=_trace and attempt == 0)
            break
        except Exception:
            if attempt == 3:
                raise
            import time
            time.sleep(5.0 * (attempt + 1))
    out = np.empty((B, T, H), dtype=np.float32)
    for c in range(N_CORES):
        yc = res.results[c]["y"]                      # [16, 128, 2048]
        yc = yc.reshape(NBLK, 128, TB, NK, BPC)       # [blk, p, tau, k, b]
        out[c * BPC:(c + 1) * BPC] = yc.transpose(4, 0, 2, 3, 1).reshape(
            BPC, T, H)
    if _trace:
        kernel._last_results = res
    return out

